# revision 86
# baseline (speedup 1.0000x reference)
"""Trainium2 Bass kernel for AttentionGuidedMaskStrategy (topk_masking).

Per batch b and side (a->mask_b, b->mask_a):
  v[j]    = sum_i qmask[i] * attn[b, i, j]           (PE, qmask broadcast to
            all 128 lhsT columns -> PSUM holds v replicated on all partitions)
  vt[p,c] = v[4p+c]                                  (4 diag ACT copies + one
            N=4 selector-transpose matmul)
  rank    = #{j : v[j] < vt[p,c]}                    (DVE fused compare+accum
            for chunks 0-2; ACT sign(vt-v) with the activation accumulator
            for chunk 3: sum_j sign() = 2*count - 511, compared against the
            host-staged integer threshold 2k-513 -- all exact)
  mask    = rank <= k-1, k = int(0.3 * n_nonpad_keys)  (exact truncation)
  out     = mask ? mask_embedding : embed            (copy_predicated)

Data parallel over 8 NeuronCores: 8 batches per core, no collectives.

Precision strategy (gate is rel err < 2e-2; this kernel measures 2.1e-4):
 - attn is staged as one packed byte tensor per side, row = [fp16 hi |
   x256-scaled fp8e4m3 residual] (3 B/elem vs f32's 4).  The lo matmul
   weights are qmask * 2^-8, so every lo product is the residual exactly
   (power-of-two scaling), and PSUM accumulates in f32.  The v sums move
   by ~1e-4 against a ~0.03 typical top-k boundary gap: the selection is
   unchanged (zero flips measured).
 - embeds and outputs move in f16 (host downcasts/upcasts): only the
   blended values round (~2e-4), never the ranking.
 - per-core HBM traffic drops 32MB (all-f32) -> 16MB, and the fp16/fp8
   matmuls run 1 PE pass per plane instead of fp32's 4 cycles/column.

All derived constants (transposed hi/lo query masks, integer mask
thresholds, broadcast mask_embedding, transpose selector) are precomputed
on the host and staged as inputs; on-chip setup is a handful of small DMAs.

Schedule: one row-side per ~3.55us, paced by the DVE chain
[3 rank chunks, mask, blend].  Software pipeline is two rows deep: row r's
vt transpose is emitted after row r+1's sum matmuls (PE never waits on the
ACT diag copies) and row r's blend after row r+1's rank chunks.  Loads ride
the sync ring, stores the gpsimd ring, so a store issue (which waits on a
blend) never delays a load issue; tile pools are 12 deep so the boot-random
hot DMA queue (it receives every DMA's completion-semaphore descriptor)
never starves the attn stream.  Layout is p-major (rows 4p..4p+3 on
partition p): every DMA is 128 contiguous per-partition runs.
"""

import sys

for _p in ("/opt/trn_rl_repo",):
    if _p not in sys.path:
        sys.path.insert(0, _p)

import numpy as np
from contextlib import ExitStack

from concourse import bacc, bass, mybir
from concourse.bass_utils import run_bass_kernel_spmd
from concourse.tile import TileContext

N_CORES = 8
B_LOC = 8      # 64 batches / 8 cores
L = 512        # La == Lb
E = 256
P = 128
NKC = L // P   # 4 chunks of 128
F32 = mybir.dt.float32
F16 = mybir.dt.float16
BF16 = mybir.dt.bfloat16
F8 = mybir.dt.float8e4   # e4m3: lo plane is pre-scaled x256 on host, weights are qmask x 2^-8
U8 = mybir.dt.uint8
OP = mybir.AluOpType

EAGER_ROWS = 3  # rows whose vt transpose is emitted un-pipelined (fill)

# packed f32 const layout: [P, 68] = kmq [2,8,4] | sel4 [4]
NC_KM = 2 * B_LOC * NKC          # 64
NC_SEL = NKC                     # 4
NCONST = NC_KM + NC_SEL          # 68


def _build() -> bass.Bass:
    nc = bacc.Bacc(None, target_bir_lowering=False)

    # attn is staged as an fp16 hi plane plus an fp8e5m2 residual plane
    # (3 bytes/element vs f32's 4): the sum matmuls run one single-cycle
    # pass per plane with exact f32 PSUM accumulation.
    # |a - hi - lo| <= ~2^-14|a|: the v sums move by ~1e-4 against a ~0.03
    # typical top-k boundary gap (measured: zero selection flips).
    # one packed byte tensor per side: row j = [fp16 hi | x256 fp8e4m3 lo],
    # so each attn row-side is ONE 768KB DMA (every dma_start's completion
    # semaphore lands on the same boot-random HW queue; fewer DMAs keep that
    # queue from becoming the pipeline's pacemaker)
    attn_a_pk = nc.declare_dram_parameter("attn_a_pk", [B_LOC, L, 3 * L], U8, isOutput=False)
    attn_b_pk = nc.declare_dram_parameter("attn_b_pk", [B_LOC, L, 3 * L], U8, isOutput=False)
    embed_a = nc.declare_dram_parameter("embed_a", [B_LOC, L, E], F16, isOutput=False)
    embed_b = nc.declare_dram_parameter("embed_b", [B_LOC, L, E], F16, isOutput=False)
    consts = nc.declare_dram_parameter("consts", [P, NCONST], F32, isOutput=False)
    qmTh = nc.declare_dram_parameter("qmTh", [P, 2, B_LOC, NKC], F16, isOutput=False)
    qmTl = nc.declare_dram_parameter("qmTl", [P, 2, B_LOC, NKC], F8, isOutput=False)
    membbc = nc.declare_dram_parameter("membbc", [P, E], F16, isOutput=False)
    out_b = nc.declare_dram_parameter("out_b", [B_LOC, L, E], F16, isOutput=True)
    out_a = nc.declare_dram_parameter("out_a", [B_LOC, L, E], F16, isOutput=True)

    with TileContext(nc) as tc, ExitStack() as ctx:
        const = ctx.enter_context(tc.tile_pool(name="const", bufs=1))
        at_pool = ctx.enter_context(tc.tile_pool(name="at", bufs=12))
        et_pool = ctx.enter_context(tc.tile_pool(name="et", bufs=12))
        scr_pool = ctx.enter_context(tc.tile_pool(name="scr", bufs=4))
        vbc_pool = ctx.enter_context(tc.tile_pool(name="vbc", bufs=4))
        rk_pool = ctx.enter_context(tc.tile_pool(name="rk", bufs=4))
        vbc_psum = ctx.enter_context(tc.tile_pool(name="vbc_ps", bufs=4, space="PSUM"))
        vt_psum = ctx.enter_context(tc.tile_pool(name="vt_ps", bufs=3, space="PSUM"))

        ones_k1 = const.tile([1, P], F32)       # lhsT for the HAM warmup
        nc.vector.memset(ones_k1[:], 1.0)
        wrow = const.tile([1, L], F32, tag="wrow")
        nc.vector.memset(wrow[:], 0.0)

        # v4 scratch (rotated by hand): v chunk kc parked on partition 32*kc;
        # all other partitions stay zero forever
        v4bufs = []
        for i in range(3):
            v4t = const.tile([P, P], F32, tag=f"v4_{i}")
            nc.vector.memset(v4t[:], 0.0)
            v4bufs.append(v4t)

        # host-staged constants: packed f32 consts, bf16 query masks, f16
        # mask embedding.  Issue order is the fill-critical order: row 0's
        # hi attn plane first (its matmuls are the longest pole), then the
        # query masks it multiplies with, then everything else -- each
        # DIRECT2D descriptor-generation costs ~0.6us of sequencer time, so
        # putting the tiny consts first would delay row 0 by ~2us.
        cst = const.tile([P, NCONST], F32, tag="cst")
        memb_sb = const.tile([P, E], F16, tag="memb")
        qmT_h = const.tile([P, 2, B_LOC, NKC], F16, tag="qmT_h")
        qmT_l = const.tile([P, 2, B_LOC, NKC], F8, tag="qmT_l")
        kmq = cst[:, 0:NC_KM].rearrange("p (s b q) -> p s b q", s=2, q=NKC)
        sel4 = cst[:, NC_KM:NCONST]

        rows0_pk = attn_a_pk[0].rearrange("(p q) c -> p q c", q=NKC)
        at0 = at_pool.tile([P, NKC, 3 * L], U8, tag="at")

        et0 = et_pool.tile([P, NKC, E], F16, tag="et")
        # two half-tile DMAs so row 0's chunk-0/1 matmuls start as soon as
        # the first 384KB lands
        nc.sync.dma_start(out=at0[:, 0:2], in_=rows0_pk[:, 0:2])
        nc.sync.dma_start(out=at0[:, 2:NKC], in_=rows0_pk[:, 2:NKC])
        cst_dma = nc.sync.dma_start(out=qmT_h[:], in_=qmTh[:, :, :, :])
        nc.sync.dma_start(out=qmT_l[:], in_=qmTl[:, :, :, :])
        nc.sync.dma_start(out=et0[:], in_=embed_b[0].rearrange(
            "(p q) e -> p q e", q=NKC))  # row 0 pairs attn_a with embed_b
        nc.sync.dma_start(out=cst[:], in_=consts[:, :])
        nc.sync.dma_start(out=memb_sb[:], in_=membbc[:, :])


        # HAM warmup: dummy PE work starting as soon as the memset operands
        # exist, so the PE clock-boost controller (needs ~7us of activity)
        # starts integrating while attn row 0 streams in.  Half-width so the
        # warmups finish right as row 0's first attn chunk lands.
        wps = vbc_psum.tile([P, L], F32, tag="vbc")
        for wi in range(2):
            nc.tensor.matmul(wps[:, :L // 2], ones_k1[:], wrow[:, :L // 2],
                             start=True, stop=True)

        # (packed attn, embed in, out, side index)
        sides = [
            (attn_a_pk, embed_b, out_b, 0),
            (attn_b_pk, embed_a, out_a, 1),
        ]
        rows = [(b,) + s for b in range(B_LOC) for s in sides]

        def emit_front(r, pending_vt=None):
            """Loads + key sums. vbc[p, j] = sum_i qmask[i] attn[i, j] on every
            partition p (qmask lhsT broadcast to all 128 columns)."""
            b, attn_pk, emb, outp, si = rows[r]
            if r == 0:
                at, et = at0, et0
            else:
                at = at_pool.tile([P, NKC, 3 * L], U8, tag="at")
                nc.sync.dma_start(
                    out=at[:], in_=attn_pk[b].rearrange("(p q) c -> p q c", q=NKC))
                et = et_pool.tile([P, NKC, E], F16, tag="et")
                nc.sync.dma_start(
                    out=et[:], in_=emb[b].rearrange("(p q) e -> p q e", q=NKC))

            # hi/lo pair per chunk, accumulating into one f32 PSUM group in
            # an order that tracks the reference's running-sum trajectory
            vbc_ps = vbc_psum.tile([P, L], F32, tag="vbc")
            for ic in range(NKC):
                nc.tensor.matmul(vbc_ps[:],
                                 qmT_h[:, si, b, ic:ic + 1].to_broadcast([P, P]),
                                 at[:, ic, 0:2 * L].bitcast(F16),
                                 start=(ic == 0), stop=False)
                nc.tensor.matmul(vbc_ps[:],
                                 qmT_l[:, si, b, ic:ic + 1].to_broadcast([P, P]),
                                 at[:, ic, 2 * L:3 * L].bitcast(F8),
                                 start=False, stop=(ic == NKC - 1))
            if pending_vt is not None:
                pending_vt()

            # v4[32*g, m] = v[4*m + g]: each psum partition already holds the
            # full v, so partition 32*g copies its own stride-4 slice
            # (ACT, psum-near engine; single-partition access needs base%32==0)
            v4 = v4bufs[r % 3]
            for g in range(NKC):
                nc.scalar.copy(
                    v4[32 * g:32 * g + 1, :],
                    vbc_ps[32 * g:32 * g + 1, :].rearrange(
                        "a (m q) -> a q m", q=NKC)[:, g])
            # bulk copy v to SBUF: DVE rank compares read SBUF at full rate
            # (PSUM-direct reads measured ~14% slower)
            vbc_sb = vbc_pool.tile([P, L], F32, tag="vbc_sb")
            nc.scalar.copy(vbc_sb[:], vbc_ps[:])
            return et, v4, vbc_sb

        def emit_back_pe(r, v4, vbc_sb):
            # vt[p, q] = v4[32*q, p] = v[4p+q] via one N=4 selector matmul
            vt_ps = vt_psum.tile([P, NKC], F32, tag="vt")
            nc.tensor.matmul(vt_ps[:], v4[:], sel4, start=True, stop=True,
                             skip_group_check=True)
            # tiny hop to SBUF so the DVE rank pass reads no PSUM operand
            vt_sb = rk_pool.tile([P, NKC], F32, tag="vt_sb")
            nc.scalar.copy(vt_sb[:], vt_ps[:])
            # ACT computes rank chunk 3 as sign(vt - v) with the activation
            # accumulator: sum_j sign(vt - v[j]) = 2*count - 511 (self gives
            # 0, values distinct; a rounded nonzero difference keeps its
            # sign, so this is exact).  Emitted HERE, right after the vt_sb
            # copy in the ACT stream, so it runs in ACT's idle window and
            # never delays the next row's diag copies or vt_sb.
            sgn = scr_pool.tile([P, L], F16, tag="sgn")
            rank4 = rk_pool.tile([P, NKC], F32, tag="rank")
            nc.scalar.activation(sgn[:], vbc_sb[:],
                                 mybir.ActivationFunctionType.Sign,
                                 bias=vt_sb[:, NKC - 1:NKC], scale=-1.0,
                                 accum_out=rank4[:, NKC - 1:NKC])
            return vt_sb, rank4

        def emit_rank(r, vbc_sb, vt_sb, rank4):
            b, attn_pk, emb, outp, si = rows[r]

            # rank[p, kc] = #{j : v[j] < vT[p, kc]}: DVE does chunks 0-2 with
            # the fused compare+accumulate (DVE-only op); chunk 3 came from
            # the ACT sign accumulator (already normalized into rank4[:, 3])
            for kc in range(NKC - 1):
                scr = scr_pool.tile([P, L], U8, tag="scr")
                nc.vector.tensor_scalar(
                    scr[:], vbc_sb[:], vt_sb[:, kc:kc + 1], None,
                    op0=OP.is_lt, op1=OP.add, accum_out=rank4[:, kc:kc + 1])

            # mask: per-column integer thresholds (cols 0-2 hold the count L,
            # threshold k-1; col 3 holds the raw sign-sum 2L-511, threshold
            # 2k-513 -- all integer-valued f32, compares exact).  On DVE: a
            # Pool-side mask costs a ~0.5us cross-engine round trip right
            # before the DVE blend, which paces the whole pipeline.
            mask4 = rk_pool.tile([P, NKC], mybir.dt.uint16, tag="mask")
            nc.vector.tensor_tensor(mask4[:], rank4[:], kmq[:, si, b],
                                    op=OP.is_le)
            return mask4

        def emit_blend(r, et, mask4):
            # blend in place: et = mask ? mask_embedding : embed, then store.
            # Emitted one row AFTER emit_rank(r) so the in-order DVE stream
            # hides the Pool mask latency behind the next row's rank chunks.
            b, attn_pk, emb, outp, si = rows[r]
            nc.vector.copy_predicated(
                et[:, :, :],
                mask4[:].unsqueeze(2).to_broadcast([P, NKC, E]),
                memb_sb[:].unsqueeze(1).to_broadcast([P, NKC, E]))
            nc.gpsimd.dma_start(
                out=outp[b].rearrange("(p q) e -> p q e", q=NKC), in_=et[:])

        # Software pipeline, two rows deep: row r's vt transpose is emitted
        # after row r+1's sum matmuls (so the PE never stalls on the ACT diag
        # copies), and row r's blend is emitted after row r+1's rank chunks
        # (so the in-order DVE stream hides the Pool mask hop).  The first
        # EAGER_ROWS rows emit their vt un-pipelined to shorten the fill.
        prev = None        # (r, et, v4, vbc_sb) awaiting rank
        pend_blend = None  # (r, et, mask4) awaiting blend
        vt_eager = {}
        for r in range(len(rows)):
            holder = {}
            pending_vt = None
            if prev is not None and prev[0] not in vt_eager:
                pr, pet, pv4, pvbc = prev

                def pending_vt(pr=pr, pv4=pv4, pvbc=pvbc, holder=holder):
                    holder["vt"] = emit_back_pe(pr, pv4, pvbc)
            state = emit_front(r, pending_vt)
            nxt_blend = None
            if prev is not None:
                pr, pet, pv4, pvbc = prev
                vt_sb, rank4 = vt_eager.get(pr) or holder["vt"]
                mask4 = emit_rank(pr, pvbc, vt_sb, rank4)
                nxt_blend = (pr, pet, mask4)
            if pend_blend is not None:
                emit_blend(*pend_blend)
            pend_blend = nxt_blend
            if r < EAGER_ROWS:
                vt_eager[r] = emit_back_pe(r, state[1], state[2])
            prev = (r,) + state
        pr, pet, pv4, pvbc = prev
        vt_sb, rank4 = vt_eager.get(pr) or emit_back_pe(pr, pv4, pvbc)
        if pend_blend is not None:
            emit_blend(*pend_blend)
        emit_blend(pr, pet, emit_rank(pr, pvbc, vt_sb, rank4))

    nc.compile()
    return nc


_NC_CACHE = None


def _get_nc() -> bass.Bass:
    global _NC_CACHE
    if _NC_CACHE is None:
        _NC_CACHE = _build()
    return _NC_CACHE


def _host_consts(a_pad, b_pad):
    """Packed per-core f32 consts [P, 20]: 0.3*len-1 thresholds and the vt
    transpose selector."""
    len_a = (~a_pad).sum(axis=1).astype(np.float32)
    len_b = (~b_pad).sum(axis=1).astype(np.float32)
    # k = int(0.3 * len) truncation, replicated exactly in f32 like the
    # reference; all staged thresholds are small integers (exact in f32).
    k_b = (np.float32(0.3) * len_b).astype(np.int32)
    k_a = (np.float32(0.3) * len_a).astype(np.int32)
    kmq = np.empty((2, B_LOC, NKC), np.float32)
    for s, k in ((0, k_b), (1, k_a)):
        # cols 0-2 compare the count L (mask iff L <= k-1); col 3 compares
        # the raw sign accumulator 2L-511 (mask iff <= 2k-513)
        kmq[s, :, 0:NKC - 1] = (k - 1).astype(np.float32)[:, None]
        kmq[s, :, NKC - 1] = (2 * k - 513).astype(np.float32)
    kmq = np.broadcast_to(kmq[None], (P, 2, B_LOC, NKC))
    sel4 = np.zeros((P, NKC), np.float32)
    for kc in range(NKC):
        sel4[32 * kc, kc] = 1.0
    out = np.empty((P, NCONST), np.float32)
    out[:, 0:NC_KM] = kmq.reshape(P, NC_KM)
    out[:, NC_KM:NCONST] = sel4
    return out


def _host_qmT(a_pad, b_pad, dt):
    """Transposed query masks [P, 2, B_LOC, NKC] (0/1, exact in any float)."""
    qa = (~a_pad).astype(np.float32).reshape(B_LOC, P, NKC).transpose(1, 0, 2)
    qb = (~b_pad).astype(np.float32).reshape(B_LOC, P, NKC).transpose(1, 0, 2)
    return np.ascontiguousarray(np.stack((qa, qb), axis=1)).astype(dt)


def _pack_hi_lo(a, f8):
    """[B, L, L] f32 -> [B, L, 3L] u8: per row, fp16 hi plane bytes then
    x256-scaled fp8e4m3 residual bytes.  The lo matmul weights are
    qmask * 2^-8, so each lo product is the residual exactly (power-of-two
    scaling); |a - hi - lo| <= ~2^-15|a|."""
    hi = a.astype(np.float16)
    lo = ((a - hi.astype(np.float32)) * np.float32(256.0)).astype(f8)
    B = a.shape[0]
    return np.ascontiguousarray(np.concatenate(
        (hi.view(np.uint8).reshape(B, L, 2 * L),
         lo.view(np.uint8).reshape(B, L, L)), axis=2))


def _run(inputs, trace=False):
    import ml_dtypes
    f8 = ml_dtypes.float8_e4m3fn
    nc = _get_nc()
    membbc = np.ascontiguousarray(np.broadcast_to(
        np.asarray(inputs["mask_embedding"]).astype(np.float16), (P, E)))
    in_maps = []
    for c in range(N_CORES):
        sl = slice(c * B_LOC, (c + 1) * B_LOC)
        a_pad = np.asarray(inputs["a_padding_mask"])[sl]
        b_pad = np.asarray(inputs["b_padding_mask"])[sl]
        in_maps.append({
            "attn_a_pk": _pack_hi_lo(np.asarray(inputs["attn_a"])[sl], f8),
            "attn_b_pk": _pack_hi_lo(np.asarray(inputs["attn_b"])[sl], f8),
            "embed_a": np.asarray(inputs["embed_a"])[sl].astype(np.float16),
            "embed_b": np.asarray(inputs["embed_b"])[sl].astype(np.float16),
            "consts": _host_consts(a_pad, b_pad),
            "qmTh": _host_qmT(a_pad, b_pad, np.float16),
            "qmTl": (_host_qmT(a_pad, b_pad, np.float32)
                     * np.float32(2.0 ** -8)).astype(f8),
            "membbc": membbc,
        })
    res = run_bass_kernel_spmd(nc, in_maps, core_ids=list(range(N_CORES)), trace=trace)
    out_b = np.concatenate(
        [res.results[c]["out_b"].astype(np.float32) for c in range(N_CORES)], axis=0)
    out_a = np.concatenate(
        [res.results[c]["out_a"].astype(np.float32) for c in range(N_CORES)], axis=0)
    return (out_b, out_a), res


def kernel(**inputs):
    outs, _ = _run(inputs, trace=False)
    return outs
